# revision 41
# baseline (speedup 1.0000x reference)
"""Trainium2 Bass kernel for the CaputoEncoder model.

Model (see reference): feats = concat([caputo(x, 0.5), caputo(x, 1.0)], -1)
-> 2-layer LSTM(512) -> last timestep -> relu(linear).

Key simplifications:
  * caputo(x, 1.0) has coefficient 1/gamma(0) == 0 -> contributes zeros;
    only the alpha=0.5 branch matters, so only Wih0[:, :250] is ever used.
  * caputo(x, .5) = d*x - Wc@x (over time) == G @ x_b with G = diag(d) - Wc,
    host-precomputed; becomes a single matmul per batch.

Sharding: pure data parallelism over batch (64 -> 8 per core, 8 cores).
All weights replicated; scatter/gather on host.

Perf structure (v2): the two LSTM layer scans are interleaved on every
core with layer-1 lagging layer-0 by one 32-step window.  A scan step is
a 1.7us PE burst (64 LDW+MM pairs at ~27ns issue) followed by a ~2.7us
serial elementwise chain during which the PE would idle; running the
other layer's burst in that shadow roughly halves the critical path.
The xw1 = A1 @ h0 + b1 input GEMM is computed per-window from the
h0 window still in SBUF (contiguous moving operand, N=256), instead of
a separate strided phase over a DRAM h0 sequence.

On-core layout (hidden-major):
  hT, cT  : (128 part = hidden%128, cols = kchunk*8 + b)   [4*8=32 cols]
  gatesT  : (128 part = gate%128,  cols = gchunk*8 + b)    [16*8=128 cols]
  gate chunks host-permuted to [i, f, o, g] so sigmoid covers cols 0..95.
"""

import math
from contextlib import ExitStack

import numpy as np
import ml_dtypes

import concourse.bass as bass
import concourse.tile as tile
from concourse import mybir
from concourse.bass import ds
from concourse.bass_utils import run_bass_kernel_spmd

AF = mybir.ActivationFunctionType
OP = mybir.AluOpType
F32 = mybir.dt.float32
BF16 = mybir.dt.bfloat16
F8 = mybir.dt.float8e4

# Whh is stored pre-scaled by 2**WHH_SHIFT; the gates STT multiplies the
# psum by 2**-WHH_SHIFT.  This lifts the uniform(+-1/sqrt(H)) weights into
# fp8-e4m3's normal range: i/f/o rows (sigmoid-squashed, error-tolerant)
# are stored e4m3 for 2x faster FWL weight loads; g rows stay bf16.
WHH_SHIFT = 12

B, T, N = 64, 512, 250
NP = 256          # n padded to 2 partition chunks
H = 512
G4 = 4 * H        # 2048
OUT = 1024
NCORES = 8
PB = B // NCORES  # 8 batches per core
WIN = 32          # scan steps per window
NWIN = T // WIN   # 16 windows

KC = H // 128     # 4 hidden chunks
GC = G4 // 128    # 16 gate chunks
NC2 = NP // 128   # 2 input chunks
CB = KC * PB      # 32 h/c columns
WB = WIN * PB     # 256 (u, b) columns per window


def _split_drain_waits(nc, max_waits=1):
    """This walrus build's CoreV3 codegen accepts at most one sem-wait per
    engine instruction (Drain/Matmult/... ISA structs have a single wait
    slot).  Move extra waits onto same-engine NoOps inserted immediately
    before the instruction — the engine blocks at the NoOp instead, which is
    semantically identical (same engine stream, same program point)."""
    for bb in nc.m.functions[0].blocks:
        insts = bb.instructions  # live list
        i = 0
        while i < len(insts):
            ins = insts[i]
            si = ins.sync_info
            if si is not None and len(si.on_wait) > max_waits:
                waits = list(si.on_wait)
                ins.sync_info = mybir.SyncInfo(
                    on_wait=waits[:max_waits], on_update=list(si.on_update)
                )
                for j, w in enumerate(waits[max_waits:]):
                    nop = mybir.InstNoOp(name=f"{ins.name}-wsplit{j}")
                    nop.engine = ins.engine
                    nop.sync_info = mybir.SyncInfo(on_wait=[w], on_update=[])
                    insts.insert(i, nop)
                    i += 1
            i += 1


class _Layer:
    """Per-layer scan state: weights, h/c slices of the shared state tiles,
    pools.  Both layers' loop-carried h/c live in ONE tile pair (cols
    [idx*CB, (idx+1)*CB)) — with two separate carried tile pairs the Tile
    For_i scheduler deadlocks (two independent loop-carried chains)."""

    def __init__(self, nc, tc, ctx, name, whh_ifo, whh_g, h_all, c_all, idx):
        self.nc = nc
        self.name = name
        self.whh_ifo = whh_ifo
        self.whh_g = whh_g
        # layer-1's SBUF elementwise chain runs on the otherwise-idle GpSimd
        # engine so the two layers' chains don't serialize on Vector.
        self.tt = nc.gpsimd if idx == 1 else nc.vector
        self.h_cur = h_all[:, idx * CB:(idx + 1) * CB]
        self.c_cur = c_all[:, idx * CB:(idx + 1) * CB]
        self.ps_pool = ctx.enter_context(
            tc.tile_pool(name=f"{name}_ps", bufs=2, space="PSUM")
        )
        self.ew_pool = ctx.enter_context(tc.tile_pool(name=f"{name}_ew", bufs=3))
        self.hw_pool = ctx.enter_context(tc.tile_pool(name=f"{name}_hw", bufs=3))


def _emit_step(l, u, win_v, hwin, h_prev, c_prev, last_of_win):
    """One LSTM step for layer `l`.

    win_v : (128, WIN, GC, PB) view of this window's gate inputs
            (bf16, pre-scaled by 2**WHH_SHIFT like the weights)
    hwin  : (128, KC*WIN*PB) bf16 tile to dump h into (layer 0) or None
    h_prev: list of KC (128, PB) APs; c_prev: (128, CB) AP
    Returns (h_prev_next, c_next).

    The gate input window is folded into the psum by an identity matmul
    per gate chunk, so the activations read PSUM directly (descaling via
    the activation's scale operand) — no psum-reading Vector op exists,
    which keeps each layer's elementwise chain off the other's engine
    FIFOs (head-of-line blocking was the dominant stall).
    """
    nc = l.nc
    psum = l.ps_pool.tile([128, GC * PB], F32, tag="ps")
    for gc in range(GC):
        for kc in range(KC):
            if gc < 12:
                lhsT = l.whh_ifo[:, kc, gc * 128:(gc + 1) * 128]
            else:
                lhsT = l.whh_g[:, kc, (gc - 12) * 128:(gc - 11) * 128]
            nc.tensor.matmul(
                psum[:, gc * PB:(gc + 1) * PB],
                lhsT,
                h_prev[kc],
                start=(kc == 0),
                stop=False,
            )
        nc.tensor.matmul(
            psum[:, gc * PB:(gc + 1) * PB],
            l.ident[:],
            win_v[:, u, gc, :],
            start=False,
            stop=True,
        )
    # i,f,o sigmoid on cols [0, 3*CB); g tanh on [3*CB, 4*CB)
    acts = l.ew_pool.tile([128, GC * PB], F32, tag="acts")
    nc.scalar.activation(acts[:, :3 * CB], psum[:, :3 * CB], AF.Sigmoid,
                         scale=2.0 ** -WHH_SHIFT)
    nc.scalar.activation(acts[:, 3 * CB:], psum[:, 3 * CB:], AF.Tanh,
                         scale=2.0 ** -WHH_SHIFT)
    # c = f*c + i*g ; h = o*tanh(c)
    ig = l.ew_pool.tile([128, CB], F32, tag="ig")
    l.tt.tensor_tensor(ig[:], acts[:, :CB], acts[:, 3 * CB:], OP.mult)
    fc = l.ew_pool.tile([128, CB], F32, tag="fc")
    l.tt.tensor_tensor(fc[:], acts[:, CB:2 * CB], c_prev, OP.mult)
    if last_of_win:
        c_new = l.c_cur
    else:
        c_tile = l.hw_pool.tile([128, CB], F32, tag="c")
        c_new = c_tile[:]
    l.tt.tensor_tensor(c_new, fc[:], ig[:], OP.add)
    tc_t = l.ew_pool.tile([128, CB], F32, tag="tc")
    nc.scalar.activation(tc_t[:], c_new, AF.Tanh)
    acts_o = acts[:, 2 * CB:3 * CB].rearrange("p (k b) -> p k b", k=KC)
    tc_v = tc_t[:].rearrange("p (k b) -> p k b", k=KC)
    if hwin is not None:
        h_out = hwin.rearrange("p (k w b) -> p w k b", k=KC, w=WIN)[:, u]
    elif last_of_win:
        h_out = l.h_cur.rearrange("p (k b) -> p k b", k=KC)
    else:
        h_tmp = l.hw_pool.tile([128, CB], BF16, tag="h")
        h_out = h_tmp[:].rearrange("p (k b) -> p k b", k=KC)
    l.tt.tensor_tensor(h_out, acts_o, tc_v, OP.mult)
    if hwin is not None and last_of_win:
        l.tt.tensor_copy(
            l.h_cur.rearrange("p (k b) -> p k b", k=KC), h_out
        )
    return [h_out[:, kc, :] for kc in range(KC)], c_new


def _emit_windows(nc, tc, pools, l0, l1, win0_src, xw1win):
    """One macro-window: layer-1 over the persistent SBUF xw1win buffer
    (content = previous window's xw1 GEMM output), layer-0 over win0_src
    (DRAM), then the xw1 GEMM from the freshly produced h0 window
    overwrites xw1win for the next iteration.

    The lagging layer (l1) is emitted FIRST each step: its psum is ready
    at the half-block, so its Vector STT never head-of-line-blocks
    layer-0's chain (l0's STT waits at the Vector head while l1's chain
    runs on ACT/GpSimd).
    """
    win_pool, gps_pool = pools
    a1_sb, b1_sb = l1.a_sb, l1.b_sb

    win0 = win_pool.tile([128, GC * WB], BF16, tag="win0")
    nc.sync.dma_start(
        win0[:].rearrange("p (g w b) -> p g w b", g=GC, w=WIN),
        win0_src.rearrange("g p w b -> p g w b"),
    )
    hwin = win_pool.tile([128, KC * WB], BF16, tag="hwin")

    sts = []
    for l, win, hw in ((l1, xw1win, None), (l0, win0, hwin)):
        win_v = win[:].rearrange("p (g w b) -> p w g b", g=GC, w=WIN)
        h_prev = [l.h_cur[:, kc * PB:(kc + 1) * PB] for kc in range(KC)]
        sts.append([l, win_v, hw, h_prev, l.c_cur])
    for u in range(WIN):
        for st in sts:
            l, win_v, hw, h_prev, c_prev = st
            h_prev, c_prev = _emit_step(
                l, u, win_v, hw, h_prev, c_prev, u == WIN - 1
            )
            st[3], st[4] = h_prev, c_prev

    # xw1 window = A1 @ h0win + b1; h0win cols (kc-major) give a
    # contiguous 256-wide moving operand per kc chunk.  Overwrites
    # xw1win in place (WAR on this window's l1 reads).
    for gc in range(GC):
        psum = gps_pool.tile([128, WB], F32, tag="gps")
        for kc in range(KC):
            nc.tensor.matmul(
                psum[:],
                a1_sb[:, kc, gc * 128:(gc + 1) * 128],
                hwin[:, kc * WB:(kc + 1) * WB],
                start=(kc == 0),
                stop=(kc == KC - 1),
            )
        nc.scalar.activation(
            xw1win[:, gc * WB:(gc + 1) * WB], psum[:], AF.Identity,
            bias=b1_sb[:, gc:gc + 1],
            scale=2.0 ** WHH_SHIFT,
        )


def build_nc():
    nc = bass.Bass()

    x_in = nc.dram_tensor("x", [PB, T, NP], BF16, kind="ExternalInput")
    gt_in = nc.dram_tensor("gt", [KC, 128, T], BF16, kind="ExternalInput")
    a0_in = nc.dram_tensor("a0t", [NC2, 128, G4], BF16, kind="ExternalInput")
    b0_in = nc.dram_tensor("b0", [128, GC], F32, kind="ExternalInput")
    whh0i_in = nc.dram_tensor("whh0i", [KC, 128, 1536], F8, kind="ExternalInput")
    whh0g_in = nc.dram_tensor("whh0g", [KC, 128, 512], BF16, kind="ExternalInput")
    a1_in = nc.dram_tensor("a1t", [KC, 128, G4], BF16, kind="ExternalInput")
    b1_in = nc.dram_tensor("b1", [128, GC], F32, kind="ExternalInput")
    whh1i_in = nc.dram_tensor("whh1i", [KC, 128, 1536], F8, kind="ExternalInput")
    whh1g_in = nc.dram_tensor("whh1g", [KC, 128, 512], BF16, kind="ExternalInput")
    ident_in = nc.dram_tensor("ident", [128, 128], F8, kind="ExternalInput")
    wout_in = nc.dram_tensor("woutt", [KC, 128, OUT], BF16, kind="ExternalInput")
    bout_in = nc.dram_tensor("boutr", [PB, OUT], F32, kind="ExternalInput")
    out_ext = nc.dram_tensor("out", [PB, OUT], F32, kind="ExternalOutput")

    # xw0 has one trailing pad window (layer-0 runs a harmless 17th window
    # while layer-1 finishes).
    xw0_dram = nc.dram_tensor("xw0s", [GC, 128, T + WIN, PB], BF16)

    with tile.TileContext(nc) as tc:
        with ExitStack() as ctx:
            const_pool = ctx.enter_context(tc.tile_pool(name="consts", bufs=1))
            state_pool = ctx.enter_context(tc.tile_pool(name="state", bufs=1))

            gt_sb = const_pool.tile([128, KC, T], BF16)
            nc.sync.dma_start(gt_sb[:], gt_in[:, :, :].rearrange("k p t -> p k t"))
            a0_sb = const_pool.tile([128, NC2, G4], BF16)
            nc.sync.dma_start(a0_sb[:], a0_in[:, :, :].rearrange("k p g -> p k g"))
            b0_sb = const_pool.tile([128, GC], F32)
            nc.sync.dma_start(b0_sb[:], b0_in[:, :])
            whh0i_sb = const_pool.tile([128, KC, 1536], F8)
            nc.sync.dma_start(whh0i_sb[:], whh0i_in[:, :, :].rearrange("k p g -> p k g"))
            whh0g_sb = const_pool.tile([128, KC, 512], BF16)
            nc.sync.dma_start(whh0g_sb[:], whh0g_in[:, :, :].rearrange("k p g -> p k g"))
            a1_sb = const_pool.tile([128, KC, G4], BF16)
            nc.sync.dma_start(a1_sb[:], a1_in[:, :, :].rearrange("k p g -> p k g"))
            b1_sb = const_pool.tile([128, GC], F32)
            nc.sync.dma_start(b1_sb[:], b1_in[:, :])
            whh1i_sb = const_pool.tile([128, KC, 1536], F8)
            nc.sync.dma_start(whh1i_sb[:], whh1i_in[:, :, :].rearrange("k p g -> p k g"))
            whh1g_sb = const_pool.tile([128, KC, 512], BF16)
            nc.sync.dma_start(whh1g_sb[:], whh1g_in[:, :, :].rearrange("k p g -> p k g"))
            ident_sb = const_pool.tile([128, 128], F8)
            nc.sync.dma_start(ident_sb[:], ident_in[:, :])
            wout_sb = const_pool.tile([128, KC, OUT], BF16)
            nc.sync.dma_start(wout_sb[:], wout_in[:, :, :].rearrange("k p g -> p k g"))
            bout_sb = const_pool.tile([PB, OUT], F32)
            nc.sync.dma_start(bout_sb[:], bout_in[:, :])

            # ---- phase A+B: featsT_b = x_bT @ G^T ; xw0 = A0 @ feats + b0 ----
            with tc.tile_pool(name="ab", bufs=2) as ab_pool, \
                 tc.tile_pool(name="abf", bufs=1) as abf_pool, \
                 tc.tile_pool(name="abps", bufs=2, space="PSUM") as abps_pool:
                feats = []
                for b in range(PB):
                    x_sb = ab_pool.tile([128, KC, NP], BF16, tag="x")
                    nc.sync.dma_start(
                        x_sb[:], x_in[b].rearrange("(k p) n -> p k n", p=128)
                    )
                    fb = abf_pool.tile([128, NC2, T], BF16, tag=f"feats{b}")
                    for mc in range(NC2):
                        psA = abps_pool.tile([128, T], F32, tag="psA")
                        for kc in range(KC):
                            nc.tensor.matmul(
                                psA[:],
                                x_sb[:, kc, mc * 128:(mc + 1) * 128],
                                gt_sb[:, kc, :],
                                start=(kc == 0),
                                stop=(kc == KC - 1),
                            )
                        nc.vector.tensor_copy(fb[:, mc, :], psA[:])
                    feats.append(fb)
                for gc in range(GC):
                    xw_sb = ab_pool.tile([128, T * PB], BF16, tag="xw")
                    xw_v = xw_sb[:].rearrange("p (t b) -> p b t", b=PB)
                    for b in range(PB):
                        psB = abps_pool.tile([128, T], F32, tag="psB")
                        for kc in range(NC2):
                            nc.tensor.matmul(
                                psB[:],
                                a0_sb[:, kc, gc * 128:(gc + 1) * 128],
                                feats[b][:, kc, :],
                                start=(kc == 0),
                                stop=(kc == NC2 - 1),
                            )
                        nc.scalar.activation(
                            xw_v[:, b, :], psB[:], AF.Identity,
                            bias=b0_sb[:, gc:gc + 1],
                            scale=2.0 ** WHH_SHIFT,
                        )
                    nc.sync.dma_start(
                        xw0_dram[gc][:, :T, :].rearrange("p t b -> p (t b)"),
                        xw_sb[:],
                    )

            # ---- interleaved dual-layer scan, layer-1 lagging one window ----
            h_all = state_pool.tile([128, 2 * CB], BF16)
            c_all = state_pool.tile([128, 2 * CB], F32)
            nc.vector.memset(h_all[:], 0.0)
            nc.vector.memset(c_all[:], 0.0)
            l0 = _Layer(nc, tc, ctx, "l0", whh0i_sb, whh0g_sb, h_all, c_all, 0)
            l1 = _Layer(nc, tc, ctx, "l1", whh1i_sb, whh1g_sb, h_all, c_all, 1)
            l0.ident = l1.ident = ident_sb
            l1.a_sb, l1.b_sb = a1_sb, b1_sb

            # layer-1's gate-input window lives in SBUF; its initial content
            # is crafted pad gates (i,f = -40 -> sigmoid 0) that pin layer-1's
            # state to exactly zero during its one-window pipeline lag.
            xw1win = state_pool.tile([128, GC * WB], BF16)
            nc.vector.memset(xw1win[:, :8 * WB], -40.0 * 2.0 ** WHH_SHIFT)
            nc.vector.memset(xw1win[:, 8 * WB:], 0.0)    # o, g gates
            # zero the xw0 tail window
            zpad = state_pool.tile([128, GC * WB], BF16)
            nc.vector.memset(zpad[:], 0.0)
            nc.sync.dma_start(
                xw0_dram[:, :, ds(T, WIN), :].rearrange("g p w b -> p g w b"),
                zpad[:].rearrange("p (g w b) -> p g w b", g=GC, w=WIN),
            )

            win_pool = ctx.enter_context(tc.tile_pool(name="wins", bufs=2))
            gps_pool = ctx.enter_context(
                tc.tile_pool(name="gps", bufs=2, space="PSUM")
            )
            pools = (win_pool, gps_pool)

            # iteration iw: layer-0 window iw, layer-1 window iw-1 (the
            # xw1win SBUF buffer holds the previous window's GEMM output).
            with tc.For_i(0, NWIN + 1, 1,
                          hint_engines=(mybir.EngineType.PE,)) as iw:
                _emit_windows(
                    nc, tc, pools, l0, l1,
                    xw0_dram[:, :, ds(iw * WIN, WIN), :],
                    xw1win,
                )

            # ---- out = relu(h1_last @ Wout.T + bout) ----
            with tc.tile_pool(name="f_ps", bufs=2, space="PSUM") as fps_pool, \
                 tc.tile_pool(name="f_o", bufs=1) as fo_pool:
                out_sb = fo_pool.tile([PB, OUT], F32)
                for half in range(2):
                    psF = fps_pool.tile([PB, 512], F32, tag="psF")
                    for kc in range(KC):
                        nc.tensor.matmul(
                            psF[:],
                            l1.h_cur[:, kc * PB:(kc + 1) * PB],
                            wout_sb[:, kc, half * 512:(half + 1) * 512],
                            start=(kc == 0),
                            stop=(kc == KC - 1),
                        )
                    sl = slice(half * 512, (half + 1) * 512)
                    nc.vector.tensor_tensor(
                        out_sb[:, sl], psF[:], bout_sb[:, sl], OP.add
                    )
                    nc.vector.tensor_scalar_max(out_sb[:, sl], out_sb[:, sl], 0.0)
                nc.sync.dma_start(out_ext[:, :], out_sb[:])

    _split_drain_waits(nc)
    return nc


_NC_CACHE = None


def _get_nc():
    global _NC_CACHE
    if _NC_CACHE is None:
        _NC_CACHE = build_nc()
    return _NC_CACHE


def _prep_host(inputs):
    x = np.asarray(inputs["x"], dtype=np.float32)
    coef = 1.0 / math.gamma(0.5)
    t = np.arange(T, dtype=np.float64)
    diff = t[:, None] - t[None, :]
    W = np.where(diff > 0, (np.abs(diff) + 1e-6) ** -0.5, 0.0).astype(np.float32)
    d = (coef * W.sum(1)).astype(np.float32)
    G = (np.diag(d) - coef * W).astype(np.float32)  # feats_b = G @ x_b
    bf = ml_dtypes.bfloat16
    GT = np.ascontiguousarray(G.T).astype(bf).reshape(KC, 128, T)

    perm = np.concatenate([  # torch gate order i,f,g,o -> [i,f,o,g]
        np.arange(0, H), np.arange(H, 2 * H),
        np.arange(3 * H, 4 * H), np.arange(2 * H, 3 * H),
    ])

    A0 = np.zeros((G4, NP), np.float32)
    A0[:, :N] = np.asarray(inputs["Wih0"], np.float32)[perm, :N]
    A0T = np.ascontiguousarray(A0.T).astype(bf).reshape(NC2, 128, G4)
    b0 = (np.asarray(inputs["bih0"], np.float32)
          + np.asarray(inputs["bhh0"], np.float32))[perm] * (2.0 ** WHH_SHIFT)
    b0_t = np.ascontiguousarray(b0.reshape(GC, 128).T)

    f8 = ml_dtypes.float8_e4m3

    def split_whh(w):
        # pre-scaled, transposed, (KC, 128, G4); i/f/o cols -> e4m3, g -> bf16
        wt = np.ascontiguousarray(
            np.asarray(w, np.float32)[perm].T * (2.0 ** WHH_SHIFT)
        ).reshape(KC, 128, G4)
        return (
            np.ascontiguousarray(wt[:, :, :1536]).astype(f8),
            np.ascontiguousarray(wt[:, :, 1536:]).astype(bf),
        )

    Whh0I, Whh0G = split_whh(inputs["Whh0"])
    Whh1I, Whh1G = split_whh(inputs["Whh1"])

    A1T = np.ascontiguousarray(
        np.asarray(inputs["Wih1"], np.float32)[perm].T
    ).astype(bf).reshape(KC, 128, G4)
    b1 = (np.asarray(inputs["bih1"], np.float32)
          + np.asarray(inputs["bhh1"], np.float32))[perm] * (2.0 ** WHH_SHIFT)
    b1_t = np.ascontiguousarray(b1.reshape(GC, 128).T)

    WoutT = np.ascontiguousarray(
        np.asarray(inputs["Wout"], np.float32).T
    ).astype(bf).reshape(KC, 128, OUT)
    bout_r = np.broadcast_to(
        np.asarray(inputs["bout"], np.float32), (PB, OUT)
    ).copy()

    xp = np.zeros((B, T, NP), bf)
    xp[:, :, :N] = x.astype(bf)

    shared = dict(
        gt=GT, a0t=A0T, b0=b0_t, whh0i=Whh0I, whh0g=Whh0G, a1t=A1T, b1=b1_t,
        whh1i=Whh1I, whh1g=Whh1G, woutt=WoutT, boutr=bout_r,
        ident=np.eye(128, dtype=np.float32).astype(f8),
    )
    in_maps = []
    for c in range(NCORES):
        m = dict(shared)
        m["x"] = np.ascontiguousarray(xp[c * PB:(c + 1) * PB])
        in_maps.append(m)
    return in_maps


def kernel(**inputs):
    nc = _get_nc()
    in_maps = _prep_host(inputs)
    res = run_bass_kernel_spmd(nc, in_maps, core_ids=list(range(NCORES)))
    out = np.concatenate([r["out"] for r in res.results], axis=0)
    return out.astype(np.float32)


# revision 48
# speedup vs baseline: 1.3674x; 1.3674x over previous
"""Trainium2 Bass kernel for the CaputoEncoder model.

Model (see reference): feats = concat([caputo(x, 0.5), caputo(x, 1.0)], -1)
-> 2-layer LSTM(512) -> last timestep -> relu(linear).

Key simplifications:
  * caputo(x, 1.0) has coefficient 1/gamma(0) == 0 -> contributes zeros;
    only the alpha=0.5 branch matters, so only Wih0[:, :250] is ever used.
  * caputo(x, .5) = d*x - Wc@x (over time) == G @ x_b with G = diag(d) - Wc,
    host-precomputed; becomes a single matmul per batch.

Sharding: pure data parallelism over batch (64 -> 8 per core, 8 cores).
All weights replicated; scatter/gather on host.

Perf structure (v2): the two LSTM layer scans are interleaved on every
core with layer-1 lagging layer-0 by one 32-step window.  A scan step is
a 1.7us PE burst (64 LDW+MM pairs at ~27ns issue) followed by a ~2.7us
serial elementwise chain during which the PE would idle; running the
other layer's burst in that shadow roughly halves the critical path.
The xw1 = A1 @ h0 + b1 input GEMM is computed per-window from the
h0 window still in SBUF (contiguous moving operand, N=256), instead of
a separate strided phase over a DRAM h0 sequence.

On-core layout (hidden-major):
  hT, cT  : (128 part = hidden%128, cols = kchunk*8 + b)   [4*8=32 cols]
  gatesT  : (128 part = gate%128,  cols = gchunk*8 + b)    [16*8=128 cols]
  gate chunks host-permuted to [i, f, o, g] so sigmoid covers cols 0..95.
"""

import math
from contextlib import ExitStack

import numpy as np
import ml_dtypes

import concourse.bass as bass
import concourse.tile as tile
from concourse import mybir
from concourse.bass import ds
from concourse.bass_utils import run_bass_kernel_spmd

AF = mybir.ActivationFunctionType
OP = mybir.AluOpType
F32 = mybir.dt.float32
BF16 = mybir.dt.bfloat16
F8 = mybir.dt.float8e4

# Whh is stored pre-scaled by 2**WHH_SHIFT; the gates STT multiplies the
# psum by 2**-WHH_SHIFT.  This lifts the uniform(+-1/sqrt(H)) weights into
# fp8-e4m3's normal range: i/f/o rows (sigmoid-squashed, error-tolerant)
# are stored e4m3 for 2x faster FWL weight loads; g rows stay bf16.
WHH_SHIFT = 12

B, T, N = 64, 512, 250
NP = 256          # n padded to 2 partition chunks
H = 512
G4 = 4 * H        # 2048
OUT = 1024
NCORES = 8
PB = B // NCORES  # 8 batches per core
WIN = 32          # scan steps per window
NWIN = T // WIN   # 16 windows

KC = H // 128     # 4 hidden chunks
GC = G4 // 128    # 16 gate chunks
NC2 = NP // 128   # 2 input chunks
CB = KC * PB      # 32 h/c columns
WB = WIN * PB     # 256 (u, b) columns per window


def _split_drain_waits(nc, max_waits=1):
    """This walrus build's CoreV3 codegen accepts at most one sem-wait per
    engine instruction (Drain/Matmult/... ISA structs have a single wait
    slot).  Move extra waits onto same-engine NoOps inserted immediately
    before the instruction — the engine blocks at the NoOp instead, which is
    semantically identical (same engine stream, same program point)."""
    for bb in nc.m.functions[0].blocks:
        insts = bb.instructions  # live list
        i = 0
        while i < len(insts):
            ins = insts[i]
            si = ins.sync_info
            if si is not None and len(si.on_wait) > max_waits:
                waits = list(si.on_wait)
                ins.sync_info = mybir.SyncInfo(
                    on_wait=waits[:max_waits], on_update=list(si.on_update)
                )
                for j, w in enumerate(waits[max_waits:]):
                    nop = mybir.InstNoOp(name=f"{ins.name}-wsplit{j}")
                    nop.engine = ins.engine
                    nop.sync_info = mybir.SyncInfo(on_wait=[w], on_update=[])
                    insts.insert(i, nop)
                    i += 1
            i += 1


class _Layer:
    """Per-layer scan state: weights, h/c slices of the shared state tiles,
    pools.  Both layers' loop-carried h/c live in ONE tile pair (cols
    [idx*CB, (idx+1)*CB)) — with two separate carried tile pairs the Tile
    For_i scheduler deadlocks (two independent loop-carried chains)."""

    def __init__(self, nc, tc, ctx, name, whh_ifo, whh_g, h_all, c_all, idx):
        self.nc = nc
        self.name = name
        self.whh_ifo = whh_ifo
        self.whh_g = whh_g
        # Both layers' SBUF elementwise chains run on the otherwise-idle
        # GpSimd engine; Vector then carries only the two psum-reading STTs,
        # whose FIFO order matches their psum-ready order (no head-of-line
        # blocking of chain ops behind a psum-waiting STT).
        self.tt = nc.gpsimd
        self.h_cur = h_all[:, idx * CB:(idx + 1) * CB]
        self.c_cur = c_all[:, idx * CB:(idx + 1) * CB]
        self.ps_pool = ctx.enter_context(
            tc.tile_pool(name=f"{name}_ps", bufs=2, space="PSUM")
        )
        self.ew_pool = ctx.enter_context(tc.tile_pool(name=f"{name}_ew", bufs=3))
        self.hw_pool = ctx.enter_context(tc.tile_pool(name=f"{name}_hw", bufs=3))


def _emit_step(l, u, win_v, hwin, h_prev, c_prev, last_of_win):
    """One LSTM step for layer `l`.

    win_v : (128, WIN, GC, PB) view of this window's gate inputs
            (bf16, pre-scaled by 2**WHH_SHIFT like the weights)
    hwin  : (128, KC*WIN*PB) bf16 tile to dump h into (layer 0) or None
    h_prev: list of KC (128, PB) APs; c_prev: (128, CB) AP
    Returns (h_prev_next, c_next).

    The gate input window is folded into the psum by an identity matmul
    per gate chunk, so the activations read PSUM directly (descaling via
    the activation's scale operand) — no psum-reading Vector op exists,
    which keeps each layer's elementwise chain off the other's engine
    FIFOs (head-of-line blocking was the dominant stall).
    """
    nc = l.nc
    psum = l.ps_pool.tile([128, GC * PB], F32, tag="ps")
    for gc in range(GC):
        for kc in range(KC):
            if gc < 12:
                lhsT = l.whh_ifo[:, kc, gc * 128:(gc + 1) * 128]
            else:
                lhsT = l.whh_g[:, kc, (gc - 12) * 128:(gc - 11) * 128]
            nc.tensor.matmul(
                psum[:, gc * PB:(gc + 1) * PB],
                lhsT,
                h_prev[kc],
                start=(kc == 0),
                stop=(kc == KC - 1),
            )
    gates = l.ew_pool.tile([128, GC * PB], F32, tag="gates")
    nc.vector.scalar_tensor_tensor(
        gates.rearrange("p (g b) -> p g b", g=GC),
        psum.rearrange("p (g b) -> p g b", g=GC),
        2.0 ** -WHH_SHIFT,
        win_v[:, u],
        OP.mult,
        OP.add,
    )
    # i,f,o sigmoid on cols [0, 3*CB); g tanh on [3*CB, 4*CB)
    acts = l.ew_pool.tile([128, GC * PB], F32, tag="acts")
    nc.scalar.activation(acts[:, :3 * CB], gates[:, :3 * CB], AF.Sigmoid)
    nc.scalar.activation(acts[:, 3 * CB:], gates[:, 3 * CB:], AF.Tanh)
    # c = f*c + i*g ; h = o*tanh(c)
    ig = l.ew_pool.tile([128, CB], F32, tag="ig")
    l.tt.tensor_tensor(ig[:], acts[:, :CB], acts[:, 3 * CB:], OP.mult)
    fc = l.ew_pool.tile([128, CB], F32, tag="fc")
    l.tt.tensor_tensor(fc[:], acts[:, CB:2 * CB], c_prev, OP.mult)
    if last_of_win:
        c_new = l.c_cur
    else:
        c_tile = l.hw_pool.tile([128, CB], F32, tag="c")
        c_new = c_tile[:]
    l.tt.tensor_tensor(c_new, fc[:], ig[:], OP.add)
    tc_t = l.ew_pool.tile([128, CB], F32, tag="tc")
    nc.scalar.activation(tc_t[:], c_new, AF.Tanh)
    acts_o = acts[:, 2 * CB:3 * CB].rearrange("p (k b) -> p k b", k=KC)
    tc_v = tc_t[:].rearrange("p (k b) -> p k b", k=KC)
    if hwin is not None:
        h_out = hwin.rearrange("p (k w b) -> p w k b", k=KC, w=WIN)[:, u]
    elif last_of_win:
        h_out = l.h_cur.rearrange("p (k b) -> p k b", k=KC)
    else:
        h_tmp = l.hw_pool.tile([128, CB], BF16, tag="h")
        h_out = h_tmp[:].rearrange("p (k b) -> p k b", k=KC)
    l.tt.tensor_tensor(h_out, acts_o, tc_v, OP.mult)
    if hwin is not None and last_of_win:
        l.tt.tensor_copy(
            l.h_cur.rearrange("p (k b) -> p k b", k=KC), h_out
        )
    return [h_out[:, kc, :] for kc in range(KC)], c_new


def _emit_windows(nc, tc, pools, l0, l1, win0_src, xw1win):
    """One macro-window: layer-1 over the persistent SBUF xw1win buffer
    (content = previous window's xw1 GEMM output), layer-0 over win0_src
    (DRAM), then the xw1 GEMM from the freshly produced h0 window
    overwrites xw1win for the next iteration.

    The lagging layer (l1) is emitted FIRST each step: its psum is ready
    at the half-block, so its Vector STT never head-of-line-blocks
    layer-0's chain (l0's STT waits at the Vector head while l1's chain
    runs on ACT/GpSimd).
    """
    win_pool, gps_pool = pools
    a1_sb, b1_sb = l1.a_sb, l1.b_sb

    win0 = win_pool.tile([128, GC * WB], BF16, tag="win0")
    nc.sync.dma_start(
        win0[:].rearrange("p (g w b) -> p g w b", g=GC, w=WIN),
        win0_src.rearrange("g p w b -> p g w b"),
    )
    hwin = win_pool.tile([128, KC * WB], BF16, tag="hwin")

    sts = []
    for l, win, hw in ((l1, xw1win, None), (l0, win0, hwin)):
        win_v = win[:].rearrange("p (g w b) -> p w g b", g=GC, w=WIN)
        h_prev = [l.h_cur[:, kc * PB:(kc + 1) * PB] for kc in range(KC)]
        sts.append([l, win_v, hw, h_prev, l.c_cur])
    for u in range(WIN):
        for st in sts:
            l, win_v, hw, h_prev, c_prev = st
            h_prev, c_prev = _emit_step(
                l, u, win_v, hw, h_prev, c_prev, u == WIN - 1
            )
            st[3], st[4] = h_prev, c_prev

    # xw1 window = A1 @ h0win + b1; h0win cols (kc-major) give a
    # contiguous 256-wide moving operand per kc chunk.  Overwrites
    # xw1win in place (WAR on this window's l1 reads).
    for gc in range(GC):
        psum = gps_pool.tile([128, WB], F32, tag="gps")
        for kc in range(KC):
            nc.tensor.matmul(
                psum[:],
                a1_sb[:, kc, gc * 128:(gc + 1) * 128],
                hwin[:, kc * WB:(kc + 1) * WB],
                start=(kc == 0),
                stop=(kc == KC - 1),
            )
        nc.scalar.activation(
            xw1win[:, gc * WB:(gc + 1) * WB], psum[:], AF.Identity,
            bias=b1_sb[:, gc:gc + 1],
        )


def build_nc():
    nc = bass.Bass()

    x_in = nc.dram_tensor("x", [PB, T, NP], BF16, kind="ExternalInput")
    gt_in = nc.dram_tensor("gt", [KC, 128, T], BF16, kind="ExternalInput")
    a0_in = nc.dram_tensor("a0t", [NC2, 128, G4], BF16, kind="ExternalInput")
    b0_in = nc.dram_tensor("b0", [128, GC], F32, kind="ExternalInput")
    whh0i_in = nc.dram_tensor("whh0i", [KC, 128, 1536], F8, kind="ExternalInput")
    whh0g_in = nc.dram_tensor("whh0g", [KC, 128, 512], BF16, kind="ExternalInput")
    a1_in = nc.dram_tensor("a1t", [KC, 128, G4], BF16, kind="ExternalInput")
    b1_in = nc.dram_tensor("b1", [128, GC], F32, kind="ExternalInput")
    whh1i_in = nc.dram_tensor("whh1i", [KC, 128, 1536], F8, kind="ExternalInput")
    whh1g_in = nc.dram_tensor("whh1g", [KC, 128, 512], BF16, kind="ExternalInput")
    ident_in = nc.dram_tensor("ident", [128, 128], F8, kind="ExternalInput")
    wout_in = nc.dram_tensor("woutt", [KC, 128, OUT], BF16, kind="ExternalInput")
    bout_in = nc.dram_tensor("boutr", [PB, OUT], F32, kind="ExternalInput")
    out_ext = nc.dram_tensor("out", [PB, OUT], F32, kind="ExternalOutput")

    # xw0 has one trailing pad window (layer-0 runs a harmless 17th window
    # while layer-1 finishes).
    xw0_dram = nc.dram_tensor("xw0s", [GC, 128, T + WIN, PB], BF16)

    with tile.TileContext(nc) as tc:
        with ExitStack() as ctx:
            const_pool = ctx.enter_context(tc.tile_pool(name="consts", bufs=1))
            state_pool = ctx.enter_context(tc.tile_pool(name="state", bufs=1))

            gt_sb = const_pool.tile([128, KC, T], BF16)
            nc.sync.dma_start(gt_sb[:], gt_in[:, :, :].rearrange("k p t -> p k t"))
            a0_sb = const_pool.tile([128, NC2, G4], BF16)
            nc.sync.dma_start(a0_sb[:], a0_in[:, :, :].rearrange("k p g -> p k g"))
            b0_sb = const_pool.tile([128, GC], F32)
            nc.sync.dma_start(b0_sb[:], b0_in[:, :])
            whh0i_sb = const_pool.tile([128, KC, 1536], F8)
            nc.sync.dma_start(whh0i_sb[:], whh0i_in[:, :, :].rearrange("k p g -> p k g"))
            whh0g_sb = const_pool.tile([128, KC, 512], BF16)
            nc.sync.dma_start(whh0g_sb[:], whh0g_in[:, :, :].rearrange("k p g -> p k g"))
            a1_sb = const_pool.tile([128, KC, G4], BF16)
            nc.sync.dma_start(a1_sb[:], a1_in[:, :, :].rearrange("k p g -> p k g"))
            b1_sb = const_pool.tile([128, GC], F32)
            nc.sync.dma_start(b1_sb[:], b1_in[:, :])
            whh1i_sb = const_pool.tile([128, KC, 1536], F8)
            nc.sync.dma_start(whh1i_sb[:], whh1i_in[:, :, :].rearrange("k p g -> p k g"))
            whh1g_sb = const_pool.tile([128, KC, 512], BF16)
            nc.sync.dma_start(whh1g_sb[:], whh1g_in[:, :, :].rearrange("k p g -> p k g"))
            ident_sb = const_pool.tile([128, 128], F8)
            nc.sync.dma_start(ident_sb[:], ident_in[:, :])
            wout_sb = const_pool.tile([128, KC, OUT], BF16)
            nc.sync.dma_start(wout_sb[:], wout_in[:, :, :].rearrange("k p g -> p k g"))
            bout_sb = const_pool.tile([PB, OUT], F32)
            nc.sync.dma_start(bout_sb[:], bout_in[:, :])

            # ---- phase A+B: featsT_b = x_bT @ G^T ; xw0 = A0 @ feats + b0 ----
            with tc.tile_pool(name="ab", bufs=2) as ab_pool, \
                 tc.tile_pool(name="abf", bufs=1) as abf_pool, \
                 tc.tile_pool(name="abps", bufs=2, space="PSUM") as abps_pool:
                feats = []
                for b in range(PB):
                    x_sb = ab_pool.tile([128, KC, NP], BF16, tag="x")
                    nc.sync.dma_start(
                        x_sb[:], x_in[b].rearrange("(k p) n -> p k n", p=128)
                    )
                    fb = abf_pool.tile([128, NC2, T], BF16, tag=f"feats{b}")
                    for mc in range(NC2):
                        psA = abps_pool.tile([128, T], F32, tag="psA")
                        for kc in range(KC):
                            nc.tensor.matmul(
                                psA[:],
                                x_sb[:, kc, mc * 128:(mc + 1) * 128],
                                gt_sb[:, kc, :],
                                start=(kc == 0),
                                stop=(kc == KC - 1),
                            )
                        nc.vector.tensor_copy(fb[:, mc, :], psA[:])
                    feats.append(fb)
                for gc in range(GC):
                    xw_sb = ab_pool.tile([128, T * PB], BF16, tag="xw")
                    xw_v = xw_sb[:].rearrange("p (t b) -> p b t", b=PB)
                    for b in range(PB):
                        psB = abps_pool.tile([128, T], F32, tag="psB")
                        for kc in range(NC2):
                            nc.tensor.matmul(
                                psB[:],
                                a0_sb[:, kc, gc * 128:(gc + 1) * 128],
                                feats[b][:, kc, :],
                                start=(kc == 0),
                                stop=(kc == NC2 - 1),
                            )
                        nc.scalar.activation(
                            xw_v[:, b, :], psB[:], AF.Identity,
                            bias=b0_sb[:, gc:gc + 1],
                        )
                    nc.sync.dma_start(
                        xw0_dram[gc][:, :T, :].rearrange("p t b -> p (t b)"),
                        xw_sb[:],
                    )

            # ---- interleaved dual-layer scan, layer-1 lagging one window ----
            h_all = state_pool.tile([128, 2 * CB], BF16)
            c_all = state_pool.tile([128, 2 * CB], F32)
            nc.vector.memset(h_all[:], 0.0)
            nc.vector.memset(c_all[:], 0.0)
            l0 = _Layer(nc, tc, ctx, "l0", whh0i_sb, whh0g_sb, h_all, c_all, 0)
            l1 = _Layer(nc, tc, ctx, "l1", whh1i_sb, whh1g_sb, h_all, c_all, 1)
            l0.ident = l1.ident = ident_sb
            l1.a_sb, l1.b_sb = a1_sb, b1_sb

            # layer-1's gate-input window lives in SBUF; its initial content
            # is crafted pad gates (i,f = -40 -> sigmoid 0) that pin layer-1's
            # state to exactly zero during its one-window pipeline lag.
            xw1win = state_pool.tile([128, GC * WB], BF16)
            nc.vector.memset(xw1win[:, :8 * WB], -40.0)  # i, f gates
            nc.vector.memset(xw1win[:, 8 * WB:], 0.0)    # o, g gates
            # zero the xw0 tail window
            zpad = state_pool.tile([128, GC * WB], BF16)
            nc.vector.memset(zpad[:], 0.0)
            nc.sync.dma_start(
                xw0_dram[:, :, ds(T, WIN), :].rearrange("g p w b -> p g w b"),
                zpad[:].rearrange("p (g w b) -> p g w b", g=GC, w=WIN),
            )

            win_pool = ctx.enter_context(tc.tile_pool(name="wins", bufs=2))
            gps_pool = ctx.enter_context(
                tc.tile_pool(name="gps", bufs=2, space="PSUM")
            )
            pools = (win_pool, gps_pool)

            # iteration iw: layer-0 window iw, layer-1 window iw-1 (the
            # xw1win SBUF buffer holds the previous window's GEMM output).
            with tc.For_i(0, NWIN + 1, 1,
                          hint_engines=(mybir.EngineType.PE,)) as iw:
                _emit_windows(
                    nc, tc, pools, l0, l1,
                    xw0_dram[:, :, ds(iw * WIN, WIN), :],
                    xw1win,
                )

            # ---- out = relu(h1_last @ Wout.T + bout) ----
            with tc.tile_pool(name="f_ps", bufs=2, space="PSUM") as fps_pool, \
                 tc.tile_pool(name="f_o", bufs=1) as fo_pool:
                out_sb = fo_pool.tile([PB, OUT], F32)
                for half in range(2):
                    psF = fps_pool.tile([PB, 512], F32, tag="psF")
                    for kc in range(KC):
                        nc.tensor.matmul(
                            psF[:],
                            l1.h_cur[:, kc * PB:(kc + 1) * PB],
                            wout_sb[:, kc, half * 512:(half + 1) * 512],
                            start=(kc == 0),
                            stop=(kc == KC - 1),
                        )
                    sl = slice(half * 512, (half + 1) * 512)
                    nc.vector.tensor_tensor(
                        out_sb[:, sl], psF[:], bout_sb[:, sl], OP.add
                    )
                    nc.vector.tensor_scalar_max(out_sb[:, sl], out_sb[:, sl], 0.0)
                nc.sync.dma_start(out_ext[:, :], out_sb[:])

    _split_drain_waits(nc)
    return nc


_NC_CACHE = None


def _get_nc():
    global _NC_CACHE
    if _NC_CACHE is None:
        _NC_CACHE = build_nc()
    return _NC_CACHE


def _prep_host(inputs):
    x = np.asarray(inputs["x"], dtype=np.float32)
    coef = 1.0 / math.gamma(0.5)
    t = np.arange(T, dtype=np.float64)
    diff = t[:, None] - t[None, :]
    W = np.where(diff > 0, (np.abs(diff) + 1e-6) ** -0.5, 0.0).astype(np.float32)
    d = (coef * W.sum(1)).astype(np.float32)
    G = (np.diag(d) - coef * W).astype(np.float32)  # feats_b = G @ x_b
    bf = ml_dtypes.bfloat16
    GT = np.ascontiguousarray(G.T).astype(bf).reshape(KC, 128, T)

    perm = np.concatenate([  # torch gate order i,f,g,o -> [i,f,o,g]
        np.arange(0, H), np.arange(H, 2 * H),
        np.arange(3 * H, 4 * H), np.arange(2 * H, 3 * H),
    ])

    A0 = np.zeros((G4, NP), np.float32)
    A0[:, :N] = np.asarray(inputs["Wih0"], np.float32)[perm, :N]
    A0T = np.ascontiguousarray(A0.T).astype(bf).reshape(NC2, 128, G4)
    b0 = (np.asarray(inputs["bih0"], np.float32)
          + np.asarray(inputs["bhh0"], np.float32))[perm]
    b0_t = np.ascontiguousarray(b0.reshape(GC, 128).T)

    f8 = ml_dtypes.float8_e4m3

    def split_whh(w):
        # pre-scaled, transposed, (KC, 128, G4); i/f/o cols -> e4m3, g -> bf16
        wt = np.ascontiguousarray(
            np.asarray(w, np.float32)[perm].T * (2.0 ** WHH_SHIFT)
        ).reshape(KC, 128, G4)
        return (
            np.ascontiguousarray(wt[:, :, :1536]).astype(f8),
            np.ascontiguousarray(wt[:, :, 1536:]).astype(bf),
        )

    Whh0I, Whh0G = split_whh(inputs["Whh0"])
    Whh1I, Whh1G = split_whh(inputs["Whh1"])

    A1T = np.ascontiguousarray(
        np.asarray(inputs["Wih1"], np.float32)[perm].T
    ).astype(bf).reshape(KC, 128, G4)
    b1 = (np.asarray(inputs["bih1"], np.float32)
          + np.asarray(inputs["bhh1"], np.float32))[perm]
    b1_t = np.ascontiguousarray(b1.reshape(GC, 128).T)

    WoutT = np.ascontiguousarray(
        np.asarray(inputs["Wout"], np.float32).T
    ).astype(bf).reshape(KC, 128, OUT)
    bout_r = np.broadcast_to(
        np.asarray(inputs["bout"], np.float32), (PB, OUT)
    ).copy()

    xp = np.zeros((B, T, NP), bf)
    xp[:, :, :N] = x.astype(bf)

    shared = dict(
        gt=GT, a0t=A0T, b0=b0_t, whh0i=Whh0I, whh0g=Whh0G, a1t=A1T, b1=b1_t,
        whh1i=Whh1I, whh1g=Whh1G, woutt=WoutT, boutr=bout_r,
        ident=np.eye(128, dtype=np.float32).astype(f8),
    )
    in_maps = []
    for c in range(NCORES):
        m = dict(shared)
        m["x"] = np.ascontiguousarray(xp[c * PB:(c + 1) * PB])
        in_maps.append(m)
    return in_maps


def kernel(**inputs):
    nc = _get_nc()
    in_maps = _prep_host(inputs)
    res = run_bass_kernel_spmd(nc, in_maps, core_ids=list(range(NCORES)))
    out = np.concatenate([r["out"] for r in res.results], axis=0)
    return out.astype(np.float32)
